# revision 4
# baseline (speedup 1.0000x reference)
"""DiscreteHazardLoss Trainium2 kernel — packed sigmoid-product stream.

Math
----
reference:  loss_b = -( sum_{j<t} log(1-h_j+eps) + [e=1] log(h_t+eps)
                        + [e=0] log(1-h_t+eps) ),  h = sigmoid(x),  mean over b.
With  log(1-h+eps) ~= -softplus(x)  (eps=1e-7 shift is ~1e-7 relative on the
mean, far below fp32 noise) and  softplus(-x) = softplus(x) - x:

    loss_b = sum_{j<=t_b} softplus(x_bj) - e_b * x_{b,t_b}

Only the MEAN over b is needed, so neither row order nor row boundaries
matter for the heavy first term: it is one global sum of softplus over the
~51.6% of elements with j <= t_b.  The host therefore packs exactly those
elements (a pure bf16 cast + gather, no arithmetic) into one flat stream
per core, padded with -30 (sigma(30) == 1.0 exactly in bf16) to a fixed
capacity.  The device then computes

    sum_i softplus(x_i) = -ln prod_i sigma(-x_i)

as: ACT sigmoid(scale=-1) full pass -> 4 DVE halving product-folds
(arbitrary pairing is fine for a global product; groups of 16 keep bf16
exponent range safe: typical group product e^-13, extreme ~e^-60 vs bf16
min normal e^-87) -> one tiny Ln+accum over the 16x-reduced data.  Only
two ACT table loads per NEFF (sigmoid set in the loop, natural_log once).

The event term sum_b e_b * x_{b,t_b} is a trivial gather of the inputs;
computed on host in float64 (as in the baseline).

Sharding: pure data-parallel over the batch axis, 8 cores; each core gets
its packed stream.  Work per core is balanced to ~0.1% by the law of large
numbers.  If a core's packed stream overflows the fixed capacity L
(>27 sigma away for uniform t; only possible for adversarial time_bins),
the overflow elements are folded in exactly on the host.
"""

import sys

for _p in ("/opt/trn_rl_repo",):
    if _p not in sys.path:
        sys.path.insert(0, _p)

import numpy as np
import ml_dtypes
from contextlib import ExitStack

import concourse.bass as bass
import concourse.bacc as bacc
import concourse.tile as tile
import concourse.mybir as mybir
from concourse.bass_utils import run_bass_kernel_spmd

B, T = 2097152, 32
NCORES = 8
P = 128                      # SBUF partitions
FD = 8528                    # free elems per partition per tile (4 | FD)
NT = 4                       # tiles per core
L = NT * P * FD              # 4,366,336 packed capacity per core (mean+8.7sigma)
ROWS_PC = B // NCORES        # 262144 rows per core
PAD = -30.0                  # sigma(30) rounds to exactly 1.0 in bf16
FOLDS = 2                    # product-fold depth: FD -> FD/4 per tile
G = FD >> FOLDS              # 2132 group-products per partition per tile

_CACHE = {}


def _build_nc(repeat=1):
    nc = bacc.Bacc(
        "TRN2",
        target_bir_lowering=False,
        debug=False,
        enable_asserts=False,
        num_devices=NCORES,
    )
    x_d = nc.dram_tensor("xp", [L], mybir.dt.bfloat16, kind="ExternalInput")
    acc_d = nc.dram_tensor("acc", [P, 1], mybir.dt.float32, kind="ExternalOutput")

    xv = x_d.ap().rearrange("(n p f) -> n p f", p=P, f=FD)   # [NT, 128, FD]

    with tile.TileContext(nc) as tc, ExitStack() as ctx:
        pool = ctx.enter_context(tc.tile_pool(name="work", bufs=3))
        singles = ctx.enter_context(tc.tile_pool(name="singles", bufs=1))

        ln_in = singles.tile([P, NT * G], mybir.dt.bfloat16)
        ln_out = singles.tile([P, NT * G], mybir.dt.bfloat16)
        acc_t = singles.tile([P, 1], mybir.dt.float32)

        for r in range(NT * repeat):
            n = r % NT
            xt = pool.tile([P, FD], mybir.dt.bfloat16, tag="x", bufs=5)
            nc.sync.dma_start(out=xt, in_=xv[n])

            # s = sigma(-x), bf16 full pass
            st = pool.tile([P, FD], mybir.dt.bfloat16, tag="s", bufs=3)
            nc.scalar.activation(
                out=st,
                in_=xt,
                func=mybir.ActivationFunctionType.Sigmoid,
                scale=-1.0,
            )

            # halving product-folds: prod of arbitrary pairs is fine
            cur, w = st, FD
            for k in range(FOLDS):
                h = w // 2
                if h == G:
                    dst = ln_in[:, n * G : (n + 1) * G]
                else:
                    dst = pool.tile([P, h], mybir.dt.bfloat16, tag=f"g{k}", bufs=2)
                nc.vector.tensor_tensor(
                    out=dst,
                    in0=cur[:, :h],
                    in1=cur[:, h:w],
                    op=mybir.AluOpType.mult,
                )
                cur, w = dst, h

        # sum_i softplus = -sum ln(group products); one small Ln + accum
        nc.scalar.activation(
            out=ln_out,
            in_=ln_in,
            func=mybir.ActivationFunctionType.Ln,
            accum_out=acc_t,
        )
        nc.sync.dma_start(out=acc_d.ap(), in_=acc_t)

    nc.compile()
    return nc


def _get_nc(repeat=1):
    key = ("nc", repeat)
    if key not in _CACHE:
        _CACHE[key] = _build_nc(repeat)
    return _CACHE[key]


def pack_inputs(logits, time_bins):
    """Host-side marshalling: bf16 cast + gather of the j<=t elements into
    per-core fixed-size streams.  Returns (list of [L] bf16 arrays,
    float64 softplus-sum of any overflow elements)."""
    logits = np.ascontiguousarray(np.asarray(logits, dtype=np.float32))
    tb = np.clip(np.asarray(time_bins).astype(np.int64), 0, T - 1).astype(np.int32)
    xb = logits.astype(ml_dtypes.bfloat16)
    keep = np.arange(T, dtype=np.int32)[None, :] <= tb[:, None]   # [B, T]

    bufs, spill_sp = [], 0.0
    for c in range(NCORES):
        sl = slice(c * ROWS_PC, (c + 1) * ROWS_PC)
        flat = xb[sl][keep[sl]]
        if flat.shape[0] > L:
            sp = flat[L:].astype(np.float64)
            spill_sp += float(
                np.sum(np.log1p(np.exp(-np.abs(sp))) + np.maximum(sp, 0.0))
            )
            flat = flat[:L]
        buf = np.full(L, PAD, dtype=ml_dtypes.bfloat16)
        buf[: flat.shape[0]] = flat
        bufs.append(buf)
    return bufs, spill_sp


def kernel(logits, time_bins, events):
    logits = np.ascontiguousarray(np.asarray(logits, dtype=np.float32))
    tb = np.clip(np.asarray(time_bins).astype(np.int64), 0, T - 1).astype(np.int32)
    events = np.asarray(events, dtype=np.int32)

    bufs, spill_sp = pack_inputs(logits, tb)
    nc = _get_nc()
    in_maps = [{"xp": bufs[c]} for c in range(NCORES)]
    res = run_bass_kernel_spmd(nc, in_maps, core_ids=list(range(NCORES)))

    total = spill_sp
    for c in range(NCORES):
        total -= res.results[c]["acc"].astype(np.float64).sum()

    # event term (tiny scalar derived from inputs; exact in float64)
    x_t = np.take_along_axis(logits, tb[:, None].astype(np.int64), axis=1)[:, 0]
    total -= float(np.where(events == 1, x_t.astype(np.float64), 0.0).sum())

    return np.float32(total / B)


# revision 7
# speedup vs baseline: 1.0010x; 1.0010x over previous
"""DiscreteHazardLoss Trainium2 kernel — packed sigmoid-product stream.

Math
----
reference:  loss_b = -( sum_{j<t} log(1-h_j+eps) + [e=1] log(h_t+eps)
                        + [e=0] log(1-h_t+eps) ),  h = sigmoid(x),  mean over b.
With  log(1-h+eps) ~= -softplus(x)  (eps=1e-7 shift is ~1e-7 relative on the
mean, far below fp32 noise) and  softplus(-x) = softplus(x) - x:

    loss_b = sum_{j<=t_b} softplus(x_bj) - e_b * x_{b,t_b}

Only the MEAN over b is needed, so neither row order nor row boundaries
matter for the heavy first term: it is one global sum of softplus over the
~51.6% of elements with j <= t_b.  The host therefore packs exactly those
elements (a pure bf16 cast + gather, no arithmetic) into one flat stream
per core, padded with -30 (sigma(30) == 1.0 exactly in bf16) to a fixed
capacity.  The device then computes

    sum_i softplus(x_i) = -ln prod_i sigma(-x_i)

as: ACT sigmoid(scale=-1) full pass -> one DVE halving product-fold
(arbitrary pairing is fine for a global product; pair products stay well
inside bf16 range) -> one Ln+accum pass over the halved data at the end.
Only two ACT table loads per NEFF (sigmoid set in the loop, natural_log
once).  Engine budget per core (measured in a quiet window): ACT sigmoid
~15.3us (2 elem/cycle/partition @1.2GHz, the binding engine), DVE fold
~9.3us (tensor_tensor bf16 2x; single fold keeps consecutive DVE ops
independent so no pipe-drain stalls), DMA ~10.1us (863 GB/s/core
measured ceiling).  fp8 input was measured 6x SLOWER (breaks the ACT
fast path) - do not revisit.

The event term sum_b e_b * x_{b,t_b} is a trivial gather of the inputs;
computed on host in float64 (as in the baseline).

Sharding: pure data-parallel over the batch axis, 8 cores; each core gets
its packed stream.  Work per core is balanced to ~0.1% by the law of large
numbers.  If a core's packed stream overflows the fixed capacity L
(>27 sigma away for uniform t; only possible for adversarial time_bins),
the overflow elements are folded in exactly on the host.
"""

import sys

for _p in ("/opt/trn_rl_repo",):
    if _p not in sys.path:
        sys.path.insert(0, _p)

import numpy as np
import ml_dtypes
from contextlib import ExitStack

import concourse.bass as bass
import concourse.bacc as bacc
import concourse.tile as tile
import concourse.mybir as mybir
from concourse.bass_utils import run_bass_kernel_spmd

B, T = 2097152, 32
NCORES = 8
P = 128                      # SBUF partitions
FD = 8528                    # free elems per partition per tile (4 | FD)
NT = 4                       # tiles per core
L = NT * P * FD              # 4,366,336 packed capacity per core (mean+8.7sigma)
ROWS_PC = B // NCORES        # 262144 rows per core
PAD = -30.0                  # sigma(30) rounds to exactly 1.0 in bf16
FOLDS = 1                    # single halving fold: consecutive DVE ops stay
                             # independent (different tiles), so no pipe-drain
                             # stalls; DVE total ~9.3us sits well under ACT
G = FD >> FOLDS              # 4264 pair-products per partition per tile

_CACHE = {}


def _build_nc(repeat=1):
    nc = bacc.Bacc(
        "TRN2",
        target_bir_lowering=False,
        debug=False,
        enable_asserts=False,
        num_devices=NCORES,
    )
    x_d = nc.dram_tensor("xp", [L], mybir.dt.bfloat16, kind="ExternalInput")
    acc_d = nc.dram_tensor("acc", [P, 1], mybir.dt.float32, kind="ExternalOutput")

    xv = x_d.ap().rearrange("(n p f) -> n p f", p=P, f=FD)   # [NT, 128, FD]

    with tile.TileContext(nc) as tc, ExitStack() as ctx:
        pool = ctx.enter_context(tc.tile_pool(name="work", bufs=3))
        singles = ctx.enter_context(tc.tile_pool(name="singles", bufs=1))

        ln_in = singles.tile([P, NT * G], mybir.dt.bfloat16)
        ln_out = singles.tile([P, NT * G], mybir.dt.bfloat16)
        acc_t = singles.tile([P, 1], mybir.dt.float32)

        for r in range(NT * repeat):
            n = r % NT
            xt = pool.tile([P, FD], mybir.dt.bfloat16, tag="x", bufs=4)
            nc.sync.dma_start(out=xt, in_=xv[n])

            # s = sigma(-x), bf16 full pass
            st = pool.tile([P, FD], mybir.dt.bfloat16, tag="s", bufs=3)
            nc.scalar.activation(
                out=st,
                in_=xt,
                func=mybir.ActivationFunctionType.Sigmoid,
                scale=-1.0,
            )

            # halving product-folds: prod of arbitrary pairs is fine
            cur, w = st, FD
            for k in range(FOLDS):
                h = w // 2
                if h == G:
                    dst = ln_in[:, n * G : (n + 1) * G]
                else:
                    dst = pool.tile([P, h], mybir.dt.bfloat16, tag=f"g{k}", bufs=2)
                nc.vector.tensor_tensor(
                    out=dst,
                    in0=cur[:, :h],
                    in1=cur[:, h:w],
                    op=mybir.AluOpType.mult,
                )
                cur, w = dst, h

        # sum_i softplus = -sum ln(group products); one small Ln + accum
        nc.scalar.activation(
            out=ln_out,
            in_=ln_in,
            func=mybir.ActivationFunctionType.Ln,
            accum_out=acc_t,
        )
        nc.sync.dma_start(out=acc_d.ap(), in_=acc_t)

    nc.compile()
    return nc


def _get_nc(repeat=1):
    key = ("nc", repeat)
    if key not in _CACHE:
        _CACHE[key] = _build_nc(repeat)
    return _CACHE[key]


def pack_inputs(logits, time_bins):
    """Host-side marshalling: bf16 cast + gather of the j<=t elements into
    per-core fixed-size streams.  Returns (list of [L] bf16 arrays,
    float64 softplus-sum of any overflow elements)."""
    logits = np.ascontiguousarray(np.asarray(logits, dtype=np.float32))
    tb = np.clip(np.asarray(time_bins).astype(np.int64), 0, T - 1).astype(np.int32)
    xb = logits.astype(ml_dtypes.bfloat16)
    keep = np.arange(T, dtype=np.int32)[None, :] <= tb[:, None]   # [B, T]

    bufs, spill_sp = [], 0.0
    for c in range(NCORES):
        sl = slice(c * ROWS_PC, (c + 1) * ROWS_PC)
        flat = xb[sl][keep[sl]]
        if flat.shape[0] > L:
            sp = flat[L:].astype(np.float64)
            spill_sp += float(
                np.sum(np.log1p(np.exp(-np.abs(sp))) + np.maximum(sp, 0.0))
            )
            flat = flat[:L]
        buf = np.full(L, PAD, dtype=ml_dtypes.bfloat16)
        buf[: flat.shape[0]] = flat
        bufs.append(buf)
    return bufs, spill_sp


def kernel(logits, time_bins, events):
    logits = np.ascontiguousarray(np.asarray(logits, dtype=np.float32))
    tb = np.clip(np.asarray(time_bins).astype(np.int64), 0, T - 1).astype(np.int32)
    events = np.asarray(events, dtype=np.int32)

    bufs, spill_sp = pack_inputs(logits, tb)
    nc = _get_nc()
    in_maps = [{"xp": bufs[c]} for c in range(NCORES)]
    res = run_bass_kernel_spmd(nc, in_maps, core_ids=list(range(NCORES)))

    total = spill_sp
    for c in range(NCORES):
        total -= res.results[c]["acc"].astype(np.float64).sum()

    # event term (tiny scalar derived from inputs; exact in float64)
    x_t = np.take_along_axis(logits, tb[:, None].astype(np.int64), axis=1)[:, 0]
    total -= float(np.where(events == 1, x_t.astype(np.float64), 0.0).sum())

    return np.float32(total / B)


# revision 12
# speedup vs baseline: 1.0311x; 1.0300x over previous
"""DiscreteHazardLoss Trainium2 kernel — packed sigmoid-product stream.

Math
----
reference:  loss_b = -( sum_{j<t} log(1-h_j+eps) + [e=1] log(h_t+eps)
                        + [e=0] log(1-h_t+eps) ),  h = sigmoid(x),  mean over b.
With  log(1-h+eps) ~= -softplus(x)  (eps=1e-7 shift is ~1e-7 relative on the
mean, far below fp32 noise) and  softplus(-x) = softplus(x) - x:

    loss_b = sum_{j<=t_b} softplus(x_bj) - e_b * x_{b,t_b}

Only the MEAN over b is needed, so neither row order nor row boundaries
matter for the heavy first term: it is one global sum of softplus over the
~51.6% of elements with j <= t_b.  The host therefore packs exactly those
elements (a pure bf16 cast + gather, no arithmetic) into one flat stream
per core, padded with -30 (sigma(30) == 1.0 exactly in bf16) to a fixed
capacity.  The device then computes

    sum_i softplus(x_i) = -ln prod_i sigma(-x_i)

as: ACT sigmoid(scale=-1) full pass -> one DVE halving product-fold
(arbitrary pairing is fine for a global product; pair products stay well
inside bf16 range) -> one Ln+accum pass over the halved data at the end.
Only two ACT table loads per NEFF (sigmoid set in the loop, natural_log
once).  Engine budget per core (measured in a quiet window): ACT sigmoid
~15.3us (2 elem/cycle/partition @1.2GHz, the binding engine), DVE fold
~9.3us (tensor_tensor bf16 2x; single fold keeps consecutive DVE ops
independent so no pipe-drain stalls), DMA ~10.1us (863 GB/s/core
measured ceiling).  fp8 input was measured 6x SLOWER (breaks the ACT
fast path) - do not revisit.

The event term sum_b e_b * x_{b,t_b} is a trivial gather of the inputs;
computed on host in float64 (as in the baseline).

Sharding: pure data-parallel over the batch axis, 8 cores; each core gets
its packed stream.  Work per core is balanced to ~0.1% by the law of large
numbers.  If a core's packed stream overflows the fixed capacity L
(mean + 8.7 sigma for uniform t; realistically only adversarial
time_bins), the overflow elements are folded in exactly on the host.
"""

import sys

for _p in ("/opt/trn_rl_repo",):
    if _p not in sys.path:
        sys.path.insert(0, _p)

import numpy as np
import ml_dtypes
from contextlib import ExitStack

import concourse.bass as bass
import concourse.bacc as bacc
import concourse.tile as tile
import concourse.mybir as mybir
from concourse.bass_utils import run_bass_kernel_spmd

B, T = 2097152, 32
NCORES = 8
P = 128                      # SBUF partitions
FD = 16896                   # free elems per partition per tile (512 | FD)
NT = 2                       # tiles per core
L = NT * P * FD              # 4,325,376 = exactly the mean packed length:
                             # zero systematic padding; overflow (half of
                             # runs, a few k elements) is folded in exactly
                             # on the host via the spill path
ROWS_PC = B // NCORES        # 262144 rows per core
PAD = -30.0                  # sigma(30) rounds to exactly 1.0 in bf16
FOLDS = 1                    # single halving fold: consecutive DVE ops stay
                             # independent (different tiles), so no pipe-drain
                             # stalls; DVE total ~9.3us sits well under ACT
G = FD >> FOLDS              # 4264 pair-products per partition per tile

_CACHE = {}


def _build_nc(repeat=1):
    nc = bacc.Bacc(
        "TRN2",
        target_bir_lowering=False,
        debug=False,
        enable_asserts=False,
        num_devices=NCORES,
    )
    x_d = nc.dram_tensor("xp", [L], mybir.dt.bfloat16, kind="ExternalInput")
    acc_d = nc.dram_tensor("acc", [P, 1], mybir.dt.float32, kind="ExternalOutput")

    xv = x_d.ap().rearrange("(n p f) -> n p f", p=P, f=FD)   # [NT, 128, FD]

    with tile.TileContext(nc) as tc, ExitStack() as ctx:
        pool = ctx.enter_context(tc.tile_pool(name="work", bufs=3))
        singles = ctx.enter_context(tc.tile_pool(name="singles", bufs=1))

        ln_in = singles.tile([P, NT * G], mybir.dt.bfloat16)
        # ln_out is write-only scratch (only the fp32 accum matters) and the
        # Ln runs once per NEFF, so fp8 storage is free SBUF savings
        ln_out = singles.tile([P, NT * G], mybir.dt.float8e5)
        acc_t = singles.tile([P, 1], mybir.dt.float32)

        for r in range(NT * repeat):
            n = r % NT
            xt = pool.tile([P, FD], mybir.dt.bfloat16, tag="x", bufs=2)
            nc.sync.dma_start(out=xt, in_=xv[n])

            # s = sigma(-x), bf16 full pass
            st = pool.tile([P, FD], mybir.dt.bfloat16, tag="s", bufs=2)
            nc.scalar.activation(
                out=st,
                in_=xt,
                func=mybir.ActivationFunctionType.Sigmoid,
                scale=-1.0,
            )

            # halving product-folds: prod of arbitrary pairs is fine
            cur, w = st, FD
            for k in range(FOLDS):
                h = w // 2
                if h == G:
                    dst = ln_in[:, n * G : (n + 1) * G]
                else:
                    dst = pool.tile([P, h], mybir.dt.bfloat16, tag=f"g{k}", bufs=2)
                nc.vector.tensor_tensor(
                    out=dst,
                    in0=cur[:, :h],
                    in1=cur[:, h:w],
                    op=mybir.AluOpType.mult,
                )
                cur, w = dst, h

        # sum_i softplus = -sum ln(group products); one small Ln + accum
        nc.scalar.activation(
            out=ln_out,
            in_=ln_in,
            func=mybir.ActivationFunctionType.Ln,
            accum_out=acc_t,
        )
        nc.sync.dma_start(out=acc_d.ap(), in_=acc_t)

    nc.compile()
    return nc


def _get_nc(repeat=1):
    key = ("nc", repeat)
    if key not in _CACHE:
        _CACHE[key] = _build_nc(repeat)
    return _CACHE[key]


def pack_inputs(logits, time_bins):
    """Host-side marshalling: bf16 cast + gather of the j<=t elements into
    per-core fixed-size streams.  Returns (list of [L] bf16 arrays,
    float64 softplus-sum of any overflow elements)."""
    logits = np.ascontiguousarray(np.asarray(logits, dtype=np.float32))
    tb = np.clip(np.asarray(time_bins).astype(np.int64), 0, T - 1).astype(np.int32)
    xb = logits.astype(ml_dtypes.bfloat16)
    keep = np.arange(T, dtype=np.int32)[None, :] <= tb[:, None]   # [B, T]

    bufs, spill_sp = [], 0.0
    for c in range(NCORES):
        sl = slice(c * ROWS_PC, (c + 1) * ROWS_PC)
        flat = xb[sl][keep[sl]]
        if flat.shape[0] > L:
            sp = flat[L:].astype(np.float64)
            spill_sp += float(
                np.sum(np.log1p(np.exp(-np.abs(sp))) + np.maximum(sp, 0.0))
            )
            flat = flat[:L]
        buf = np.full(L, PAD, dtype=ml_dtypes.bfloat16)
        buf[: flat.shape[0]] = flat
        bufs.append(buf)
    return bufs, spill_sp


def kernel(logits, time_bins, events):
    logits = np.ascontiguousarray(np.asarray(logits, dtype=np.float32))
    tb = np.clip(np.asarray(time_bins).astype(np.int64), 0, T - 1).astype(np.int32)
    events = np.asarray(events, dtype=np.int32)

    bufs, spill_sp = pack_inputs(logits, tb)
    nc = _get_nc()
    in_maps = [{"xp": bufs[c]} for c in range(NCORES)]
    res = run_bass_kernel_spmd(nc, in_maps, core_ids=list(range(NCORES)))

    total = spill_sp
    for c in range(NCORES):
        total -= res.results[c]["acc"].astype(np.float64).sum()

    # event term (tiny scalar derived from inputs; exact in float64)
    x_t = np.take_along_axis(logits, tb[:, None].astype(np.int64), axis=1)[:, 0]
    total -= float(np.where(events == 1, x_t.astype(np.float64), 0.0).sum())

    return np.float32(total / B)
